# revision 16
# baseline (speedup 1.0000x reference)
"""Trainium2 Bass kernel for the raw-reshape RoPE attention problem.

Math structure (verified against the reference):
  The reference reshapes [B, N, H*D] -> [B, H, N, D] with a *raw* reshape
  (no transpose): head h only sees input tokens [h*128, (h+1)*128) and the
  output rows [h*128, (h+1)*128) depend only on head h.  The 32 (b, h)
  pairs are fully independent: 4 pairs per NeuronCore, no collectives.
  Within a head, rows are permuted j' = s*128 + t (s = weight block,
  t = token); softmax/attention are invariant to that permutation.  RoPE
  even/odd channels are pre-permuted on the host so the rotation is
  contiguous-block arithmetic (cancels inside q.k).

SINGLE-TILING-MODE DESIGN.  Switching the PE between tiling modes (e.g.
(64,128) row-tiled vs (128,128) full) drains the array at ~250ns per
switch, so EVERY matmul in this kernel is shaped (row=64, col=128) and
runs on PE tiles (0,0)/(64,0), which execute CONCURRENTLY:

  scores   lhsT = k-chunk channels [64, 128keys]; tile T0 computes the
           even chunk while T8 computes the odd chunk (2x the effective
           rate of the K=64-contraction scores).  q channels duplicated
           across both partition halves (SBUF->SBUF DMA).
  AV       contraction (128 keys) split: T0 takes keys 0:63 -> accA,
           T8 keys 64:127 -> accB (different PSUM banks as required for
           row tiles); normalize adds accA+accB on DVE.
  proj     contraction (128 chans per k-chunk) split the same way:
           T0 -> ppA, T8 -> ppB; the A+B add folds into rope (q/k), the
           vsb copy (v), and the osb copy (wo) as DVE tensor_adds.
  transp   out = x.T via row-tiled matmuls against identity blocks
           ident[0:64,0:64] / ident[64:128,64:128] (token halves ->
           separate PSUM banks), NOT transpose-mode (which would be a
           mode switch).

  E = exp(s/8) on ScalarE over flat [128, 1024] PSUM (two chunks: cols
  0:512 even, 512:1024 odd).  Softmax sums ride column 0 of vsb
  ([1 | zeros(63) | V]) into acc partition 0.  acc is QUARTER-sized
  ([128, 4s, 128t]; 4 quarters per pair) so accA+accB fit in 2 PSUM
  banks: psS 2x2 + psA 2 + psW 2 = 8 banks.

Schedule: pair-level software pipeline as before -- proj/rope/transposes
of pair p+1 and the deferred wo-projection of pair p-1 are emitted inside
pair p's attention, landing in the PE slack of the ScalarE-bound exp
stream.
"""

import numpy as np
import ml_dtypes

N_CORES = 8
B, N, DIM = 2, 2048, 1024
H, HD = 16, 64
PAIRS_PER_CORE = 4
BF16 = ml_dtypes.bfloat16

_CACHE = {}


def _rope_tables():
    """cos/sin tables [128 t, 16 s, 32 i]; (t=0, s=0) is the unrotated row."""
    inv = 1.0 / (10000.0 ** (np.arange(0, HD, 2, dtype=np.float32) / HD))
    pos = np.arange(128 * 16, dtype=np.float32).reshape(128, 16) - 1.0  # j-1
    ang = pos[:, :, None] * inv[None, None, :]          # [128, 16, 32]
    c = np.cos(ang).astype(np.float32)
    s = np.sin(ang).astype(np.float32)
    c[0, 0, :] = 1.0
    s[0, 0, :] = 0.0
    return c, s


def _chan_perm():
    """c = s*64 + 2i + par -> c' = s*64 + par*32 + i."""
    perm = np.zeros(DIM, np.int64)
    for s in range(16):
        i = np.arange(32)
        perm[s * 64 + i] = s * 64 + 2 * i
        perm[s * 64 + 32 + i] = s * 64 + 2 * i + 1
    return perm


def _build_nc():
    import concourse.mybir as mybir
    import concourse.tile as tile
    from concourse import bacc
    from concourse.masks import make_identity

    dt = mybir.dt
    AF = mybir.ActivationFunctionType

    nc = bacc.Bacc("TRN2", target_bir_lowering=False, debug=False,
                   num_devices=N_CORES)

    xt_d = nc.declare_dram_parameter("xt", [PAIRS_PER_CORE, 128, 8, 128],
                                     dt.bfloat16, isOutput=False)
    w_d = {}
    for name in ("wq", "wk", "wv", "wo"):
        w_d[name] = nc.declare_dram_parameter(name + "t", [128, 8, 1024],
                                              dt.bfloat16, isOutput=False)
    rc_d = nc.declare_dram_parameter("ropec", [128, 16, 32], dt.float32,
                                     isOutput=False)
    rs_d = nc.declare_dram_parameter("ropes", [128, 16, 32], dt.float32,
                                     isOutput=False)
    out_d = nc.declare_dram_parameter("out", [PAIRS_PER_CORE, 128, 1024],
                                      dt.float32, isOutput=True)

    with tile.TileContext(nc) as tc:
        with (
            tc.tile_pool(name="wts", bufs=1) as wts,
            tc.tile_pool(name="const", bufs=1) as constp,
            tc.tile_pool(name="xin", bufs=4) as xin,
            tc.tile_pool(name="tmp", bufs=4) as tmpp,
            tc.tile_pool(name="qkr", bufs=3) as qkrp,
            tc.tile_pool(name="vsb", bufs=1) as vsbp,
            tc.tile_pool(name="qkt", bufs=1) as qktp,
            tc.tile_pool(name="esb", bufs=8) as esbp,
            tc.tile_pool(name="rsb", bufs=2) as rsbp,
            tc.tile_pool(name="vtm", bufs=2) as vtmp_p,
            tc.tile_pool(name="osb", bufs=2) as osbp,
            tc.tile_pool(name="psA", bufs=1, space="PSUM") as psA,
            tc.tile_pool(name="psS", bufs=2, space="PSUM") as psS,
            tc.tile_pool(name="psW", bufs=2, space="PSUM") as psW,
        ):
            xs0 = xin.tile([128, 8, 128], dt.bfloat16, tag="xt")
            nc.sync.dma_start(xs0[:], xt_d[0])
            rc = constp.tile([128, 16, 32], dt.float32, tag="rc")
            rs = constp.tile([128, 16, 32], dt.float32, tag="rs")
            nc.scalar.dma_start(rc[:], rc_d[:])
            nc.scalar.dma_start(rs[:], rs_d[:])
            # touch Exp immediately so ACT_TABLE_LOAD runs during DMA warmup
            warm = constp.tile([1, 1], dt.float32, tag="warm")
            nc.scalar.activation(warm[:], rc[0:1, 0:1, 0:1], AF.Exp)
            ident = constp.tile([128, 128], dt.bfloat16, tag="id")
            make_identity(nc, ident[:])
            w_sb = {}
            dma_eng = {"wq": nc.sync, "wk": nc.scalar,
                       "wv": nc.sync, "wo": nc.scalar}
            for name in ("wq", "wk", "wv", "wo"):
                t = wts.tile([128, 8, 1024], dt.bfloat16, tag=name)
                for kk in range(8):
                    dma_eng[name].dma_start(t[:, kk, :], w_d[name][:, kk, :])
                w_sb[name] = t

            # persistent q2t/k2t/vsb (double-buffered by pair parity).
            # q2t holds the 64 q channels DUPLICATED across partition
            # halves; k2t interleaves even chunks on 0:64, odd on 64:128;
            # vsb columns [ones | zeros(63) | V] (sums -> acc partition 0).
            q_tiles, k_tiles, v_tiles = [], [], []
            for vi in range(2):
                qt = qktp.tile([128, 16, 128], dt.bfloat16, tag=f"q2t{vi}",
                               name=f"q2t_{vi}")
                kt = qktp.tile([128, 8, 128], dt.bfloat16, tag=f"k2t{vi}",
                               name=f"k2t_{vi}")
                vt = vsbp.tile([128, 16, 128], dt.bfloat16, tag=f"v{vi}",
                               name=f"vsb{vi}")
                nc.gpsimd.memset(vt[:, :, 1:64], 0.0)
                nc.vector.memset(vt[:, :, 0:1], 1.0)
                q_tiles.append(qt)
                k_tiles.append(kt)
                v_tiles.append(vt)

            fin_work = []   # deferred output projection of the previous pair

            def emit_fin(otok_t, p_idx):
                # wo projection, contraction split T0/T8 -> finA/finB;
                # the A+B add replaces the old psum-evacuation copy.
                osb = osbp.tile([128, 1024], dt.float32, tag="osb")
                for nt in range(2):
                    finA = psW.tile([128, 512], dt.float32, tag="w",
                                    name=f"finA{p_idx}_{nt}")
                    finB = psW.tile([128, 512], dt.float32, tag="w",
                                    name=f"finB{p_idx}_{nt}")
                    cs = slice(nt * 512, (nt + 1) * 512)
                    for kk in range(8):
                        nc.tensor.matmul(
                            finA[:], otok_t[0:64, kk, :],
                            w_sb["wo"][0:64, kk, cs],
                            start=(kk == 0), stop=(kk == 7))
                    for kk in range(8):
                        nc.tensor.matmul(
                            finB[:], otok_t[64:128, kk, :],
                            w_sb["wo"][64:128, kk, cs],
                            start=(kk == 0), stop=(kk == 7))
                    nc.vector.tensor_copy(osb[:, cs], finA[:])
                    nc.vector.tensor_add(osb[:, cs], osb[:, cs], finB[:])
                nc.sync.dma_start(out_d[p_idx], osb[:])

            def rope_emit(ppA, ppB, dst, sh):
                """rope with the T0+T8 partial-sum fold on DVE.  DVE has a
                single PSUM read port, so ppA is evacuated to SBUF first
                and each add reads one PSUM + one SBUF operand."""
                ca = tmpp.tile([128, 8, 2, 32], dt.float32, tag="ca")
                nc.vector.tensor_copy(ca[:], ppA)
                xe = tmpp.tile([128, 8, 32], dt.float32, tag="xe")
                xo = tmpp.tile([128, 8, 32], dt.float32, tag="xo")
                nc.vector.tensor_add(xe[:], ca[:, :, 0, :], ppB[:, :, 0, :])
                nc.vector.tensor_add(xo[:], ca[:, :, 1, :], ppB[:, :, 1, :])
                t1 = tmpp.tile([128, 8, 32], dt.float32, tag="t1")
                t2 = tmpp.tile([128, 8, 32], dt.float32, tag="t2")
                nc.vector.tensor_mul(t1[:], xe[:], rc[:, sh, :])
                nc.vector.tensor_mul(t2[:], xo[:], rs[:, sh, :])
                nc.vector.tensor_sub(dst[:, sh, 0, :], t1[:], t2[:])
                t3 = tmpp.tile([128, 8, 32], dt.float32, tag="t1")
                t4 = tmpp.tile([128, 8, 32], dt.float32, tag="t2")
                nc.vector.tensor_mul(t3[:], xe[:], rs[:, sh, :])
                nc.vector.tensor_mul(t4[:], xo[:], rc[:, sh, :])
                nc.vector.tensor_add(dst[:, sh, 1, :], t3[:], t4[:])

            def proj_transp(p):
                """projections + rope + transposes for pair p; returns
                (q2t, k2t, vsb) ready for attention."""
                if p == 0:
                    xs = xs0
                else:
                    xs = xin.tile([128, 8, 128], dt.bfloat16, tag="xt",
                                  name=f"xs{p}")
                    nc.sync.dma_start(xs[:], xt_d[p])

                q2t = q_tiles[p % 2]
                k2t = k_tiles[p % 2]
                vsb = v_tiles[p % 2]

                qr = qkrp.tile([128, 16, 2, 32], dt.bfloat16, tag="qr",
                               name=f"qr{p}")
                kr = qkrp.tile([128, 16, 2, 32], dt.bfloat16, tag="kr",
                               name=f"kr{p}")

                def chain(wname, dstA, dstB, cs):
                    for kk in range(8):
                        nc.tensor.matmul(
                            dstA, xs[0:64, kk, :], w_sb[wname][0:64, kk, cs],
                            start=(kk == 0), stop=(kk == 7))
                    for kk in range(8):
                        nc.tensor.matmul(
                            dstB, xs[64:128, kk, :],
                            w_sb[wname][64:128, kk, cs],
                            start=(kk == 0), stop=(kk == 7))

                def proj_qk(tname, dst):
                    for nt in range(2):
                        sh = slice(nt * 8, (nt + 1) * 8)
                        ppA = psW.tile([128, 8, 2, 32], dt.float32, tag="w",
                                       name=f"ppA{p}_{tname}_{nt}")
                        ppB = psW.tile([128, 8, 2, 32], dt.float32, tag="w",
                                       name=f"ppB{p}_{tname}_{nt}")
                        chain(tname, ppA[:], ppB[:],
                              slice(nt * 512, (nt + 1) * 512))
                        rope_emit(ppA[:], ppB[:], dst, sh)

                if p == 0:
                    # prologue: wq chains in psS ([128,2,...]: T0 bank0 /
                    # T8 bank1), wk-nt0 in psW; round-robin by k-chunk so
                    # weight chunks are consumed on arrival.  wk-nt1 runs
                    # after wk-nt0's rope frees the psW slots.
                    pq = {}
                    for nt in range(2):
                        pq[nt] = psS.tile([128, 2, 8, 2, 32], dt.float32,
                                          tag="s", name=f"pq0_{nt}")
                    pkA = psW.tile([128, 8, 2, 32], dt.float32, tag="w",
                                   name="pk0A")
                    pkB = psW.tile([128, 8, 2, 32], dt.float32, tag="w",
                                   name="pk0B")
                    for kk in range(8):
                        for nt in range(2):
                            cs = slice(nt * 512, (nt + 1) * 512)
                            nc.tensor.matmul(
                                pq[nt][:, 0, :, :, :], xs[0:64, kk, :],
                                w_sb["wq"][0:64, kk, cs],
                                start=(kk == 0), stop=(kk == 7))
                            nc.tensor.matmul(
                                pq[nt][:, 1, :, :, :], xs[64:128, kk, :],
                                w_sb["wq"][64:128, kk, cs],
                                start=(kk == 0), stop=(kk == 7))
                        nc.tensor.matmul(
                            pkA[:], xs[0:64, kk, :],
                            w_sb["wk"][0:64, kk, 0:512],
                            start=(kk == 0), stop=(kk == 7))
                        nc.tensor.matmul(
                            pkB[:], xs[64:128, kk, :],
                            w_sb["wk"][64:128, kk, 0:512],
                            start=(kk == 0), stop=(kk == 7))
                    for nt in range(2):
                        sh = slice(nt * 8, (nt + 1) * 8)
                        rope_emit(pq[nt][:, 0, :, :, :], pq[nt][:, 1, :, :, :],
                                  qr, sh)
                    rope_emit(pkA[:], pkB[:], kr, slice(0, 8))
                    # wk nt1 chain (weights resident by now)
                    pkA2 = psW.tile([128, 8, 2, 32], dt.float32, tag="w",
                                    name="pk1A")
                    pkB2 = psW.tile([128, 8, 2, 32], dt.float32, tag="w",
                                    name="pk1B")
                    chain("wk", pkA2[:], pkB2[:], slice(512, 1024))
                    rope_emit(pkA2[:], pkB2[:], kr, slice(8, 16))
                else:
                    proj_qk("wq", qr)
                    proj_qk("wk", kr)

                # transposes as ROW-TILED matmuls against identity blocks:
                # T0 transposes tokens 0:63 -> tpA, T8 tokens 64:127 -> tpB
                # (adjacent psW slots = different banks).  Each handles an
                # s-PAIR: tp partitions 0:64 = s_even chans, 64:128 = odd.
                def transp_pair(src, j):
                    tpA = psW.tile([128, 64], dt.float32, tag="w",
                                   name=f"tpA{p}_{j}")
                    tpB = psW.tile([128, 64], dt.float32, tag="w",
                                   name=f"tpB{p}_{j}")
                    nc.tensor.matmul(
                        tpA[:], src[0:64, 2 * j:2 * j + 2, :, :],
                        ident[0:64, 0:64], start=True, stop=True)
                    nc.tensor.matmul(
                        tpB[:], src[64:128, 2 * j:2 * j + 2, :, :],
                        ident[64:128, 64:128], start=True, stop=True)
                    return tpA, tpB

                for j in range(8):
                    tpA, tpB = transp_pair(qr, j)
                    for sub in range(2):
                        s = 2 * j + sub
                        nc.vector.tensor_copy(
                            q2t[0:64, s, 0:64], tpA[sub * 64:(sub + 1) * 64, :])
                        nc.vector.tensor_copy(
                            q2t[0:64, s, 64:128],
                            tpB[sub * 64:(sub + 1) * 64, :])
                    if j == 3:
                        nc.sync.dma_start(q2t[64:128, 0:8, :],
                                          q2t[0:64, 0:8, :])
                nc.sync.dma_start(q2t[64:128, 8:16, :], q2t[0:64, 8:16, :])
                for j in range(8):
                    tpA, tpB = transp_pair(kr, j)
                    nc.vector.tensor_copy(k2t[:, j, 0:64], tpA[:])
                    nc.vector.tensor_copy(k2t[:, j, 64:128], tpB[:])

                # V projection, contraction split T0/T8; A+B add replaces
                # the old psum-evacuation copy
                for nt in range(2):
                    sh = slice(nt * 8, (nt + 1) * 8)
                    vA = psW.tile([128, 8, 64], dt.float32, tag="w",
                                  name=f"vA{p}_{nt}")
                    vB = psW.tile([128, 8, 64], dt.float32, tag="w",
                                  name=f"vB{p}_{nt}")
                    chain("wv", vA[:], vB[:], slice(nt * 512, (nt + 1) * 512))
                    nc.vector.tensor_copy(vsb[:, sh, 64:128], vA[:])
                    nc.vector.tensor_add(vsb[:, sh, 64:128],
                                         vsb[:, sh, 64:128], vB[:])
                return q2t, k2t, vsb

            def att_quarter(p, qh, tiles, otok, mid=None, late=None):
                """one quarter = 4 s-blocks (512 q cols) x all 16 chunks."""
                q2t, k2t, vsb = tiles
                accA = psA.tile([128, 4, 128], dt.float32, tag="accA",
                                name=f"accA{p}_{qh}")
                accB = psA.tile([128, 4, 128], dt.float32, tag="accB",
                                name=f"accB{p}_{qh}")
                s0 = qh * 4

                def av(e, i):
                    for ci in range(2):
                        c = 2 * i + ci
                        cs = slice(ci * 512, (ci + 1) * 512)
                        nc.tensor.matmul(
                            accA[:], vsb[0:64, c, :], e[0:64, cs],
                            start=(c == 0), stop=(c == 15),
                            skip_group_check=True)
                        nc.tensor.matmul(
                            accB[:], vsb[64:128, c, :], e[64:128, cs],
                            start=(c == 0), stop=(c == 15),
                            skip_group_check=True)

                pend = []
                for i in range(8):      # chunk-pair index
                    sct = psS.tile([128, 1024], dt.float32, tag="s",
                                   name=f"sct{p}_{qh}_{i}")
                    # row-tiled scores: even chunk on T0 -> cols 0:512
                    # (bank n), odd chunk on T8 -> cols 512:1024 (bank n+1)
                    nc.tensor.matmul(
                        sct[:, 0:512],
                        k2t[0:64, i, :],
                        q2t[0:64, s0:s0 + 4, :],
                        start=True, stop=True)
                    nc.tensor.matmul(
                        sct[:, 512:1024],
                        k2t[64:128, i, :],
                        q2t[64:128, s0:s0 + 4, :],
                        start=True, stop=True)
                    e = esbp.tile([128, 1024], dt.bfloat16, tag="e")
                    nc.scalar.activation(e[:], sct[:], AF.Exp, scale=0.125)
                    pend.append((e, i))
                    if len(pend) > 2:
                        av(*pend.pop(0))
                    if i == 2 and mid is not None:
                        mid()
                    if i == 5 and late is not None:
                        late()
                while pend:
                    av(*pend.pop(0))

                # normalize: accA + accB (keys split) -- accB is evacuated
                # to SBUF first (DVE has one PSUM read port), and the V-row
                # sum lands on partitions 0:64 so the otok muls read both
                # SBUF operands from the same start partition (walrus
                # requires SBUF inputs of tensor_tensor to align; PSUM
                # operands are exempt).
                cB = vtmp_p.tile([128, 4, 128], dt.float32, tag="cB")
                nc.vector.tensor_copy(cB[:], accB[:])
                sums = rsbp.tile([1, 4, 128], dt.float32, tag="sum")
                nc.vector.tensor_add(sums[:], accA[0:1, :, :],
                                     cB[0:1, :, :])
                asumV = vtmp_p.tile([64, 4, 128], dt.float32, tag="vt")
                nc.vector.tensor_add(asumV[:], accA[64:128, :, :],
                                     cB[64:128, :, :])
                rsb = rsbp.tile([1, 4, 128], dt.float32, tag="r")
                nc.vector.reciprocal_approx_fast(out=rsb[:], in_=sums[:])
                rbc = rsbp.tile([64, 4, 128], dt.float32, tag="rbc")
                nc.gpsimd.partition_broadcast(rbc[:], rsb[:])
                # even s-blocks -> otok rows 0:64, odd -> 64:128
                # (s = qh*4 + sl; otok free idx g = s//2 = qh*2 + sl//2)
                for par in range(2):
                    nc.vector.tensor_mul(
                        otok[par * 64:par * 64 + 64,
                             qh * 2:qh * 2 + 2, :],
                        asumV[:, par:4:2, :],
                        rbc[:, par:4:2, :])

            # pair-level software pipeline: proj/transp of pair p+1 and
            # fin of pair p-1 are emitted inside pair p's 3rd quarter
            tiles = proj_transp(0)
            nxt_box = [None]
            for p in range(PAIRS_PER_CORE):
                otok = osbp.tile([128, 8, 128], dt.bfloat16, tag="otok",
                                 name=f"otok{p}")

                def mid(p=p):
                    if p + 1 < PAIRS_PER_CORE:
                        nxt_box[0] = proj_transp(p + 1)

                def late():
                    while fin_work:
                        emit_fin(*fin_work.pop(0))

                for qh in range(4):
                    att_quarter(p, qh, tiles, otok,
                                mid=mid if qh == 2 else None,
                                late=late if qh == 2 else None)
                fin_work.append((otok, p))
                tiles = nxt_box[0]

            while fin_work:
                emit_fin(*fin_work.pop(0))

    nc.compile()
    return nc



def _get_nc():
    if "nc" not in _CACHE:
        _CACHE["nc"] = _build_nc()
    return _CACHE["nc"]


def _prep_inputs(x, wq, wk, wv, wo):
    perm = _chan_perm()
    ropec, ropes = _rope_tables()

    def wt(w):
        # [out_chan, dim] -> transposed, partition-major [128, 8, 1024]
        return np.ascontiguousarray(
            w.T.reshape(8, 128, DIM).transpose(1, 0, 2)).astype(BF16)

    wqt = wt(wq[perm, :])
    wkt = wt(wk[perm, :])
    wvt = wt(wv)
    wot = wt(wo)

    in_maps = []
    for core in range(N_CORES):
        xts = np.empty((PAIRS_PER_CORE, 128, 8, 128), BF16)
        for pl in range(PAIRS_PER_CORE):
            pg = core * PAIRS_PER_CORE + pl
            b, h = pg // H, pg % H
            X = x[b, h * 128:(h + 1) * 128, :]      # [128 tok, 1024]
            xts[pl] = np.ascontiguousarray(
                X.T.reshape(8, 128, 128).transpose(1, 0, 2)).astype(BF16)
        in_maps.append({
            "xt": xts,
            "wqt": wqt, "wkt": wkt, "wvt": wvt, "wot": wot,
            "ropec": ropec, "ropes": ropes,
        })
    return in_maps


def run_sharded(x, wq, wk, wv, wo, trace=False, **run_kwargs):
    """Build + run on 8 cores; returns (full_output, BassKernelResults)."""
    from concourse.bass_utils import run_bass_kernel_spmd

    nc = _get_nc()
    in_maps = _prep_inputs(np.asarray(x, np.float32), np.asarray(wq, np.float32),
                           np.asarray(wk, np.float32), np.asarray(wv, np.float32),
                           np.asarray(wo, np.float32))
    res = run_bass_kernel_spmd(nc, in_maps, core_ids=list(range(N_CORES)),
                               trace=trace, **run_kwargs)
    out = np.empty((B, N, DIM), np.float32)
    for core in range(N_CORES):
        o = np.asarray(res.results[core]["out"], np.float32)
        for pl in range(PAIRS_PER_CORE):
            pg = core * PAIRS_PER_CORE + pl
            b, h = pg // H, pg % H
            out[b, h * 128:(h + 1) * 128, :] = o[pl]
    return out, res


def kernel(x, wq, wk, wv, wo):
    out, _ = run_sharded(x, wq, wk, wv, wo, trace=False)
    return out
